# revision 2
# baseline (speedup 1.0000x reference)
"""LlamaFastMLP (quantized SwiGLU MLP) Trainium2 kernel, tensor-parallel over 8 cores.

Strategy:
  - Shard INTER (11008 -> padded 11264 = 8*1408) across 8 cores; x replicated.
  - Matmul raw 4-bit codes (shipped int8, exact in bf16) against bf16 x; scales
    applied by dequantizing W tiles on DVE against PE-broadcast scale rows
    (ones-column x scale-row K=1 matmul into PSUM); zero-points and biases are
    folded into the same PSUM accumulation as extra contraction rows:
        g = x~ @ (q*s).T + xg @ (-z*s).T + 1*bias
    with xg[t,gh] = per-hidden-group sums of x~, computed on PE with selector
    matrices so each group's sum lands on its own PSUM partition.  The down
    projection uses the identical construction with groups along INTER.
  - Each core computes a partial y.T [4096, 2048]; host sums the 8 partials.
"""
import sys
sys.path.insert(0, "/opt/trn_rl_repo")

import numpy as np
import ml_dtypes

BF16 = np.dtype(ml_dtypes.bfloat16)

H = 4096          # HIDDEN
S = 2048          # tokens
G = 128           # quant group size
NCORES = 8
IPAD = 11264      # INTER padded to 8*1408
IPC = IPAD // NCORES   # 1408 per core
NIB = IPC // 128  # 11 i-blocks per core
NGH = H // G      # 32 hidden groups
TBS = 512         # token block
NTB = S // TBS    # 4
NHDS = 8          # hd slices of 512
DG = NIB          # down groups per core (along INTER) = 11

_compiled = {}


def _build():
    import concourse.bacc as bacc
    import concourse.mybir as mybir
    from concourse.tile import TileContext

    nc = bacc.Bacc("TRN2", target_bir_lowering=False, debug=False,
                   num_devices=NCORES)
    dt = mybir.dt
    f32, bf16, i8 = dt.float32, dt.bfloat16, dt.int8

    xT = nc.declare_dram_parameter("xT", [NGH, 128, S], bf16, isOutput=False)
    gq = nc.declare_dram_parameter("gq", [NIB, NGH, 128, 128], i8, isOutput=False)
    uq = nc.declare_dram_parameter("uq", [NIB, NGH, 128, 128], i8, isOutput=False)
    dq = nc.declare_dram_parameter("dq", [NHDS, NIB, 128, 512], i8, isOutput=False)
    sg = nc.declare_dram_parameter("sg", [NGH, IPC], bf16, isOutput=False)
    su = nc.declare_dram_parameter("su", [NGH, IPC], bf16, isOutput=False)
    sd = nc.declare_dram_parameter("sd", [DG, H], bf16, isOutput=False)
    zsg = nc.declare_dram_parameter("zsg", [NGH + 1, IPC], bf16, isOutput=False)
    zsu = nc.declare_dram_parameter("zsu", [NGH + 1, IPC], bf16, isOutput=False)
    zsd = nc.declare_dram_parameter("zsd", [DG + 1, H], bf16, isOutput=False)
    yT = nc.declare_dram_parameter("yT", [H, S], f32, isOutput=True)

    hscr = nc.dram_tensor("hscr", [NIB, 128, S], bf16)

    AF = mybir.ActivationFunctionType

    with TileContext(nc) as tc:
        with tc.tile_pool(name="glob", bufs=1) as gp:
            # onesrow: [1,128] of 1.0 for K=1 broadcast matmuls
            onesrow = gp.tile([1, 128], bf16)
            nc.vector.memset(onesrow[:], 1.0)
            # selector buffer: selB[:, 127] = 1, else 0.
            # selB[:, 127-r : 127-r+M] is a [128, M] matrix whose column r is
            # ones -> matmul with it sums partitions into output row r.
            selB = gp.tile([128, 256], bf16)
            nc.vector.memset(selB[:], 0.0)
            nc.vector.memset(selB[:, 127:128], 1.0)
            xgX = gp.tile([NGH + 1, S], bf16)      # rows 0-31 xg, row 32 ones
            nc.vector.memset(xgX[NGH:NGH + 1, :], 1.0)
            hgX = gp.tile([DG + 1, S], bf16)       # row 0 ones, rows 1-11 hg

            # ---------------- gate/up phase ----------------
            with tc.tile_pool(name="guc", bufs=1) as cp, \
                 tc.tile_pool(name="xp", bufs=1) as xp, \
                 tc.tile_pool(name="wp", bufs=1) as wp, \
                 tc.tile_pool(name="qp", bufs=1) as qp, \
                 tc.tile_pool(name="sp", bufs=1) as sp, \
                 tc.tile_pool(name="hp", bufs=1) as hp, \
                 tc.tile_pool(name="pp", bufs=1, space="PSUM") as pp:

                zsg_sb = cp.tile([NGH + 1, IPC], bf16)
                nc.sync.dma_start(zsg_sb[:], zsg[:])
                zsu_sb = cp.tile([NGH + 1, IPC], bf16)
                nc.sync.dma_start(zsu_sb[:], zsu[:])

                x_sb = []
                for gh in range(NGH):
                    xt = xp.tile([128, S], bf16, name=f"x{gh}")
                    nc.sync.dma_start(xt[:], xT[gh])
                    x_sb.append(xt)

                # xg group sums: selector matmuls accumulate into rows 0..31
                for tb in range(NTB):
                    ts = slice(tb * TBS, (tb + 1) * TBS)
                    xgp = pp.tile([NGH, TBS], f32, tag="ps", bufs=4)
                    for gh in range(NGH):
                        nc.tensor.matmul(xgp[:], selB[:, 127 - gh:127 - gh + NGH],
                                         x_sb[gh][:, ts],
                                         start=(gh == 0), stop=(gh == NGH - 1))
                    nc.scalar.copy(xgX[0:NGH, ts], xgp[:])

                for ib in range(NIB):
                    io = ib * 128
                    wts = {"g": [], "u": []}
                    for m, qsrc, ssrc in (("g", gq, sg), ("u", uq, su)):
                        for gh in range(NGH):
                            qt = qp.tile([128, 128], i8, tag="qt", bufs=8)
                            nc.sync.dma_start(qt[:], qsrc[ib, gh])
                            st = sp.tile([1, 128], bf16, tag="st", bufs=8)
                            nc.sync.dma_start(st[:], ssrc[gh:gh + 1, io:io + 128])
                            bcp = pp.tile([128, 128], f32, tag="bc", bufs=2)
                            nc.tensor.matmul(bcp[:], onesrow[:], st[:])
                            wt = wp.tile([128, 128], bf16, tag="wt", bufs=144)
                            nc.vector.tensor_mul(wt[:], qt[:], bcp[:])
                            wts[m].append(wt)
                    for tb in range(NTB):
                        ts = slice(tb * TBS, (tb + 1) * TBS)
                        psg = pp.tile([128, TBS], f32, tag="ps", bufs=4)
                        nc.tensor.matmul(psg[:], zsg_sb[:, io:io + 128],
                                         xgX[:, ts], start=True, stop=False)
                        for gh in range(NGH):
                            nc.tensor.matmul(psg[:], wts["g"][gh][:],
                                             x_sb[gh][:, ts],
                                             start=False, stop=(gh == NGH - 1))
                        psu = pp.tile([128, TBS], f32, tag="ps", bufs=4)
                        nc.tensor.matmul(psu[:], zsu_sb[:, io:io + 128],
                                         xgX[:, ts], start=True, stop=False)
                        for gh in range(NGH):
                            nc.tensor.matmul(psu[:], wts["u"][gh][:],
                                             x_sb[gh][:, ts],
                                             start=False, stop=(gh == NGH - 1))
                        a = hp.tile([128, TBS], bf16, tag="a", bufs=4)
                        nc.scalar.activation(a[:], psg[:], AF.Silu)
                        ht = hp.tile([128, TBS], bf16, tag="ht", bufs=4)
                        nc.vector.tensor_mul(ht[:], a[:], psu[:])
                        nc.sync.dma_start(hscr[ib, :, ts], ht[:])

            # ---------------- down phase ----------------
            with tc.tile_pool(name="dc", bufs=1) as dc, \
                 tc.tile_pool(name="hp2", bufs=1) as hp2, \
                 tc.tile_pool(name="wp2", bufs=1) as wp2, \
                 tc.tile_pool(name="qp2", bufs=1) as qp2, \
                 tc.tile_pool(name="sp2", bufs=1) as sp2, \
                 tc.tile_pool(name="yp", bufs=1) as yp, \
                 tc.tile_pool(name="pp2", bufs=1, space="PSUM") as pp2:

                zsd_sb = dc.tile([DG + 1, H], bf16)
                nc.sync.dma_start(zsd_sb[:], zsd[:])
                h_sb = []
                for ib in range(NIB):
                    hrt = hp2.tile([128, S], bf16, name=f"h{ib}")
                    nc.sync.dma_start(hrt[:], hscr[ib])
                    h_sb.append(hrt)

                # hg group sums into rows 1..11; row 0 set to ones after copy
                for tb in range(NTB):
                    ts = slice(tb * TBS, (tb + 1) * TBS)
                    hgp = pp2.tile([DG + 1, TBS], f32, tag="hgp", bufs=2)
                    for ib in range(NIB):
                        r = ib + 1
                        nc.tensor.matmul(hgp[:], selB[:, 127 - r:127 - r + DG + 1],
                                         h_sb[ib][:, ts],
                                         start=(ib == 0), stop=(ib == NIB - 1))
                    nc.scalar.copy(hgX[0:DG + 1, ts], hgp[:])
                    nc.vector.memset(hgX[0:1, ts], 1.0)

                for hds in range(NHDS):
                    ho = hds * 512
                    wds = []
                    for ib in range(NIB):
                        qd = qp2.tile([128, 512], i8, tag="qd", bufs=6)
                        nc.sync.dma_start(qd[:], dq[hds, ib])
                        std = sp2.tile([1, 512], bf16, tag="std", bufs=6)
                        nc.sync.dma_start(std[:], sd[ib:ib + 1, ho:ho + 512])
                        bcd = pp2.tile([128, 512], f32, tag="bcd", bufs=2)
                        nc.tensor.matmul(bcd[:], onesrow[:], std[:])
                        wd = wp2.tile([128, 512], bf16, tag="wd", bufs=24)
                        nc.vector.tensor_mul(wd[:], qd[:], bcd[:])
                        wds.append(wd)
                    for hsub in range(4):
                        co = ho + hsub * 128
                        for tb in range(NTB):
                            ts = slice(tb * TBS, (tb + 1) * TBS)
                            psy = pp2.tile([128, TBS], f32, tag="psy", bufs=4)
                            nc.tensor.matmul(psy[:], zsd_sb[:, co:co + 128],
                                             hgX[:, ts], start=True, stop=False)
                            for ib in range(NIB):
                                nc.tensor.matmul(
                                    psy[:],
                                    wds[ib][:, hsub * 128:(hsub + 1) * 128],
                                    h_sb[ib][:, ts],
                                    start=False, stop=(ib == NIB - 1))
                            ys = yp.tile([128, TBS], f32, tag="ys", bufs=4)
                            nc.scalar.copy(ys[:], psy[:])
                            nc.sync.dma_start(yT[co:co + 128, ts], ys[:])

    nc.compile()
    return nc


def _prep(x, gate_qw, gate_qz, gate_scale, gate_bias,
          up_qw, up_qz, up_scale, up_bias,
          down_qw, down_qz, down_scale, down_bias):
    """Host-side marshalling: shard, transpose, block, cast."""
    xT = np.ascontiguousarray(np.asarray(x, np.float32).reshape(S, H).T) \
        .astype(BF16).reshape(NGH, 128, S)

    def pad_rows(a, n):
        out = np.zeros((n,) + a.shape[1:], a.dtype)
        out[:a.shape[0]] = a
        return out

    maps = [dict(xT=xT) for _ in range(NCORES)]

    for name, qw, qz, sc, bias in (("g", gate_qw, gate_qz, gate_scale, gate_bias),
                                   ("u", up_qw, up_qz, up_scale, up_bias)):
        qw = pad_rows(np.asarray(qw, np.int32), IPAD).astype(np.int8)
        sc = pad_rows(np.asarray(sc, np.float32), IPAD)
        qz = pad_rows(np.asarray(qz, np.int32), IPAD).astype(np.float32)
        bias = pad_rows(np.asarray(bias, np.float32).reshape(-1, 1), IPAD)[:, 0]
        zs = -(qz * sc)                                    # [IPAD, 32]
        for c in range(NCORES):
            rs = slice(c * IPC, (c + 1) * IPC)
            qb = np.ascontiguousarray(qw[rs].T).reshape(NGH, 128, NIB, 128) \
                .transpose(2, 0, 1, 3).copy()              # [NIB, NGH, 128, 128]
            sT = np.ascontiguousarray(sc[rs].T).astype(BF16)   # [32, IPC]
            zsX = np.concatenate([zs[rs].T, bias[rs][None, :]], 0).astype(BF16)
            maps[c][f"{name}q"] = qb
            maps[c][f"s{name}"] = sT
            maps[c][f"zs{name}"] = zsX

    dqw = np.asarray(down_qw, np.int32).astype(np.int8)     # [H, 11008]
    dqw = np.concatenate([dqw, np.zeros((H, IPAD - 11008), np.int8)], 1)
    dsc = np.asarray(down_scale, np.float32)
    dsc = np.concatenate([dsc, np.zeros((H, IPAD // G - 86), np.float32)], 1)
    dqz = np.asarray(down_qz, np.int32).astype(np.float32)
    dqz = np.concatenate([dqz, np.zeros((H, IPAD // G - 86), np.float32)], 1)
    dzs = -(dqz * dsc)                                      # [H, 88]
    for c in range(NCORES):
        rs = slice(c * IPC, (c + 1) * IPC)
        gsl = slice(c * DG, (c + 1) * DG)
        qb = np.ascontiguousarray(dqw[:, rs].T).reshape(NIB, 128, NHDS, 512) \
            .transpose(2, 0, 1, 3).copy()                   # [NHDS, NIB, 128, 512]
        maps[c]["dq"] = qb
        maps[c]["sd"] = np.ascontiguousarray(dsc[:, gsl].T).astype(BF16)
        zrow = np.zeros((1, H), np.float32)                 # down bias added on host
        maps[c]["zsd"] = np.concatenate([zrow, dzs[:, gsl].T], 0).astype(BF16)
    return maps


def kernel(**inputs):
    from concourse.bass_utils import run_bass_kernel_spmd

    if "nc" not in _compiled:
        _compiled["nc"] = _build()
    nc = _compiled["nc"]

    maps = _prep(
        inputs["x"], inputs["gate_qw"], inputs["gate_qz"], inputs["gate_scale"],
        inputs["gate_bias"], inputs["up_qw"], inputs["up_qz"], inputs["up_scale"],
        inputs["up_bias"], inputs["down_qw"], inputs["down_qz"],
        inputs["down_scale"], inputs["down_bias"])

    res = run_bass_kernel_spmd(nc, maps, list(range(NCORES)))
    _compiled["last_results"] = res

    acc = np.zeros((H, S), np.float64)
    for c in range(NCORES):
        acc += res.results[c]["yT"].astype(np.float64)
    y = acc.T.astype(np.float32) + np.asarray(inputs["down_bias"], np.float32)[None, :]
    return y.reshape(1, S, H)
